# revision 5
# baseline (speedup 1.0000x reference)
"""Trainium2 Bass kernel for nn_CMAF (cross-modal attention fusion block).

Layout: feature-major activations on-chip — every tile is
[128 features (partitions) x 1024 samples (free)], so all matmuls are
weight-stationary bf16 with the batch as the moving free dimension.
Inputs are pre-cast to bf16 host-side and loaded straight into
feature-major SBUF via DMA-transpose (2-byte xbar path).

Engine-balance design (v3):
 - 2-way attention softmax = sigmoid((s0-s1)*ISQ) computed as
   0.5 + 0.5*tanh(d*ISQ/2); the 0.5 factors and the out-projection are
   folded into the weights host-side:
     o = ow @ (th (*) dv') + OVH @ P[s0] + OVH @ P[s1] + I @ P[n]
   with dv' = (0.5 Wv) dP, OVH = ow (0.5 Wv).  Residual adds ride the
   PSUM accumulator (identity matmuls) so the PSUM->SBUF landing is a
   single ScalarE cast with the bias folded in.
 - LayerNorm variance streams are packed [3, BLK]: three selector-column
   matmuls accumulate the per-branch variances into one PSUM tile, one
   Ln + one Exp (rsqrt via the natural_log_exp table set) serve all
   three branches, and tiny matmuls broadcast 1/sigma back to 128
   partitions.
 - ACT table sets: per-tick emission groups the gelu/tanh ops (set G)
   ahead of the ln/exp ops (set E) to bound table reloads at ~2/tick.
 - FFN runs on materialized x1, so Gelu reads its matmul PSUM directly.

LayerNorm mean subtraction is folded into the weights host-side
(centering matrix C = I - 11^T/128 on each producing linear layer).

Data parallel over 8 NeuronCores: 8192 samples each.
"""

import numpy as np
import ml_dtypes

import concourse.bass as bass
import concourse.mybir as mybir
from concourse.tile import TileContext
from concourse.vector_clock import ScopedClock
from concourse.bass_utils import run_bass_kernel_spmd

F32 = mybir.dt.float32
BF16 = mybir.dt.bfloat16
AL = mybir.AluOpType
AF = mybir.ActivationFunctionType
NPBF = ml_dtypes.bfloat16

D = 128
SP = 1280
FFN = 256
NB = 3
DH = 32
KV_IDX = ((1, 2), (0, 2), (0, 1))
NCORES = 8
BLK = 1024
MMN = 512
EPS = 1e-5
ISQ = float(1.0 / np.sqrt(DH))


def _patch_tile_drain():
    """walrus here rejects >4 sem waits on one instruction; Tile's tail
    drain carries one wait per logical proc.  Re-emit them as standalone
    wait_ge instructions ahead of the drain."""
    TC = TileContext
    if getattr(TC, "_drain_patched", False):
        return

    def patched(self, tick_clock, wait_clock):
        nop_inst = self.nc.sync.nop()
        wait_clock.add_sem_waits(
            nop_inst.ins, ScopedClock({None: tick_clock.global_clock})
        )
        d = nop_inst.ins
        si = d.sync_info
        waits = list(si.on_wait) if si is not None else []
        if len(waits) > 4:
            si.on_wait = []
            d.sync_info = si
            name2sem = {s.name: s for s in self.sems.allocated().values()}
            for w in waits:
                sem = name2sem.get(w.ant_name)
                if sem is None:
                    raise RuntimeError(f"drain patch: unknown sem {w.ant_name}")
                self.nc.sync.wait_ge(sem, w.wait_value)
        self.nc.sync.drain()
        self.nc.all_engine_barrier()
        popped = self.nc._tile_sem_poison_stack.pop()
        assert popped is self._sem_poison
        self.nc.clear_and_free_semaphores(list(self.sems.allocated().values()))
        self.nc.all_engine_barrier()

    TC._drain_and_barrier = patched
    TC._drain_patched = True


def _fix_wait_overflow(nc):
    """walrus enforces per-opcode caps on sync-wait commands attached to
    one instruction (DmaTransposeAnt: 1, others: ~4).  Move the excess
    onto same-engine NOPs inserted immediately before the instruction."""
    LIMITS = {}
    DEFAULT_LIM = 1
    for fn in nc.m.functions:
        for bb in fn.blocks:
            insts = list(bb.instructions)
            out = []
            changed = False
            for inst in insts:
                si = getattr(inst, "sync_info", None)
                w = list(si.on_wait) if si is not None and si.on_wait else []
                lim = LIMITS.get(type(inst).__name__, DEFAULT_LIM)
                if len(w) > lim:
                    excess = w[lim:]
                    keep = w[:lim]
                    eng = nc.engines[inst.engine]
                    nops = []
                    for i in range(0, len(excess), 1):
                        chunk = excess[i:i + 1]
                        nop_bi = eng.nop()
                        nop_inst = nop_bi.ins
                        cb = nc.cur_bb.bb
                        cb.instructions = [x for x in cb.instructions
                                           if x.name != nop_inst.name]
                        import bass_rust
                        nop_inst.sync_info = bass_rust.SyncInfo(
                            on_wait=chunk, on_update=[])
                        nops.append(nop_inst)
                    si.on_wait = keep
                    inst.sync_info = si
                    out.extend(nops)
                    changed = True
                out.append(inst)
            if changed:
                bb.instructions = out


def prep_weights(inp):
    """Host-side prep of all weights into SBUF layouts. bf16 for matmul
    operands, fp32 for per-partition bias vectors."""
    f64 = np.float64
    C = np.eye(D, dtype=f64) - 1.0 / D

    def bf(a):
        return np.ascontiguousarray(a.astype(np.float32)).astype(NPBF)

    def f32(a):
        return np.ascontiguousarray(a, dtype=np.float32)

    w = {}
    wsp = C @ inp["proj_w_spatial"].astype(f64)            # [128,1280]
    w["wspT"] = bf(np.transpose(wsp.reshape(D, 10, D), (2, 1, 0)).reshape(D, 10 * D))
    wgf = np.stack([C @ inp["proj_w_gf"][i].astype(f64) for i in range(2)])
    w["wgfT"] = bf(np.transpose(wgf, (2, 0, 1)).reshape(D, 2 * D))
    w["bc"] = f32(C @ inp["proj_b"].astype(f64).T)         # [128,3]
    w["emb"] = f32(inp["mod_emb"].T)

    ipw = inp["in_proj_w"].astype(f64)                     # [3, 384, 128]
    wq, wk, wv = ipw[:, :D], ipw[:, D:2 * D], ipw[:, 2 * D:]
    w["wqT"] = bf(np.transpose(wq, (2, 0, 1)).reshape(D, NB * D))
    w["wkT"] = bf(np.transpose(wk, (2, 0, 1)).reshape(D, NB * D))
    wvh = 0.5 * wv                                         # fold softmax 0.5
    w["wvhT"] = bf(np.transpose(wvh, (2, 0, 1)).reshape(D, NB * D))
    ow = np.stack([C @ inp["out_proj_w"][n].astype(f64) for n in range(NB)])
    w["owT"] = bf(np.transpose(ow, (2, 0, 1)).reshape(D, NB * D))
    ovh = np.stack([ow[n] @ wvh[n] for n in range(NB)])    # ow (0.5 Wv)
    w["ovhT"] = bf(np.transpose(ovh, (2, 0, 1)).reshape(D, NB * D))
    ob2 = np.stack([
        C @ inp["out_proj_b"][n].astype(f64)
        - inp["mod_emb"][n].astype(f64).mean()
        for n in range(NB)])
    w["ob2"] = f32(ob2.T)

    w1 = inp["ffn_w1"].astype(f64)                         # [3, 256, 128]
    w["w1T"] = bf(np.transpose(w1, (2, 0, 1)).reshape(D, NB * FFN))
    w["b1"] = f32(inp["ffn_b1"].reshape(NB * 2, D).T)      # [128, 6]
    w2 = np.stack([C @ inp["ffn_w2"][n].astype(f64) for n in range(NB)])
    w2c = w2.reshape(NB, D, 2, D)                          # [n, j, c, p]
    w["w2T"] = bf(np.transpose(w2c, (3, 0, 2, 1)).reshape(D, NB * 2 * D))
    b2c = np.stack([C @ inp["ffn_b2"][n].astype(f64) for n in range(NB)])
    w["b2c"] = f32(b2c.T)

    gw = inp["gate_w"].astype(f64).reshape(NB, NB, D)      # [j, n, p]
    w["gwT"] = bf(np.transpose(gw, (2, 1, 0)).reshape(D, NB * NB))
    w["gateb"] = f32(inp["gate_b"].reshape(NB, 1))

    # stats selector: column n = 1/D -> variance of branch n lands on
    # PSUM partition n (3 accumulating matmuls share one [3, BLK] tile)
    st = np.zeros((D, NB * NB), dtype=np.float32)
    for n in range(NB):
        st[:, NB * n + n] = 1.0 / D
    w["statsT"] = bf(st)
    # broadcast selector: row n ones -> rb3[n] broadcast to 128 partitions
    bsel = np.zeros((NB, NB * D), dtype=np.float32)
    for n in range(NB):
        bsel[n, n * D:(n + 1) * D] = 1.0
    w["bselT"] = bf(bsel)

    hs = np.zeros((D, D), dtype=np.float32)
    for h in range(4):
        hs[h * DH:(h + 1) * DH, h * DH:(h + 1) * DH] = 1.0
    w["hsel"] = bf(hs)
    w["ones3"] = bf(np.ones((NB, D)))
    esel = np.zeros((NB, NB * D), dtype=np.float32)
    for n in range(NB):
        esel[n, n * D:(n + 1) * D] = 1.0
    w["esel"] = bf(esel)
    w["ident"] = bf(np.eye(D))
    w["epsv"] = np.full((D, 1), EPS, dtype=np.float32)
    w["zerov"] = np.zeros((D, 1), dtype=np.float32)

    assert np.allclose(inp["proj_ln_g"], 1) and np.allclose(inp["proj_ln_b"], 0)
    assert np.allclose(inp["attn_ln_g"], 1) and np.allclose(inp["attn_ln_b"], 0)
    assert np.allclose(inp["ffn_ln_g"], 1) and np.allclose(inp["ffn_ln_b"], 0)
    assert np.allclose(inp["in_proj_b"], 0)
    return w


WEIGHT_SPECS = {
    "wspT": ((D, 10 * D), BF16), "wgfT": ((D, 2 * D), BF16),
    "bc": ((D, NB), F32), "emb": ((D, NB), F32),
    "wqT": ((D, NB * D), BF16), "wkT": ((D, NB * D), BF16),
    "wvhT": ((D, NB * D), BF16), "owT": ((D, NB * D), BF16),
    "ovhT": ((D, NB * D), BF16),
    "ob2": ((D, NB), F32),
    "w1T": ((D, NB * FFN), BF16), "b1": ((D, NB * 2), F32),
    "w2T": ((D, NB * 2 * D), BF16), "b2c": ((D, NB), F32),
    "gwT": ((D, NB * NB), BF16), "gateb": ((NB, 1), F32),
    "statsT": ((D, NB * NB), BF16), "bselT": ((NB, NB * D), BF16),
    "hsel": ((D, D), BF16),
    "ones3": ((NB, D), BF16), "esel": ((NB, NB * D), BF16),
    "ident": ((D, D), BF16),
    "epsv": ((D, 1), F32), "zerov": ((D, 1), F32),
}


def build_program(Bc, repeat=1):
    nc = bass.Bass()
    xsp = nc.dram_tensor("x_spatial", [Bc, SP], BF16, kind="ExternalInput")
    xg = nc.dram_tensor("x_gradient", [Bc, D], BF16, kind="ExternalInput")
    xf = nc.dram_tensor("x_frequency", [Bc, D], BF16, kind="ExternalInput")
    wd = {k: nc.dram_tensor(k, list(s[0]), s[1], kind="ExternalInput")
          for k, s in WEIGHT_SPECS.items()}
    out = nc.dram_tensor("out", [Bc, D], F32, kind="ExternalOutput")

    nblk = Bc // BLK
    assert Bc % BLK == 0

    with TileContext(nc) as tc, nc.allow_low_precision(reason="bf16 kernel"):
        with (
            tc.tile_pool(name="wp", bufs=1) as wp,
            tc.tile_pool(name="xin", bufs=2) as xin,
            tc.tile_pool(name="work", bufs=2) as wk_,
            tc.tile_pool(name="outp", bufs=1) as outp,
            tc.tile_pool(name="ps", bufs=4, space="PSUM") as psp,
        ):
            W = {}
            for k, s in WEIGHT_SPECS.items():
                W[k] = wp.tile(list(s[0]), s[1], tag=k, name=k)
                nc.gpsimd.dma_start(W[k][:], wd[k][:])
            ident = W["ident"]

            def mm(out_ap, lhsT, rhs, start=True, stop=True):
                for h in range(BLK // MMN):
                    nc.tensor.matmul(out_ap[:, h * MMN:(h + 1) * MMN], lhsT,
                                     rhs[:, h * MMN:(h + 1) * MMN],
                                     start=start, stop=stop)

            def ln_pack(sqs, tag):
                """sqs: 3 bf16 [D, BLK] squared tiles -> per-branch 1/sigma
                broadcast tiles ([D, BLK] f32 PSUM), lazily per branch."""
                var3 = psp.tile([NB, BLK], F32, tag="ps")
                for n in range(NB):
                    mm(var3[:], W["statsT"][:, NB * n:NB * (n + 1)], sqs[n][:],
                       start=(n == 0), stop=(n == NB - 1))
                lnv = wk_.tile([NB, BLK], F32, tag="lnv", bufs=1)
                nc.scalar.activation(lnv[:], var3[:], AF.Ln,
                                     bias=W["epsv"][:NB, 0:1])
                rb3 = wk_.tile([NB, BLK], BF16, tag="rb3", bufs=2)
                nc.scalar.activation(rb3[:], lnv[:], AF.Exp, scale=-0.5,
                                     bias=W["zerov"][:NB, 0:1])

                def bcast(n):
                    rb_ps = psp.tile([D, BLK], F32, tag="ps")
                    mm(rb_ps[:], W["bselT"][:NB, n * D:(n + 1) * D], rb3[:])
                    return rb_ps
                return bcast

            def phase0(b):
                r0 = (b % nblk) * BLK
                st = {}
                xspT_all = xin.tile([D, 10 * BLK], BF16, tag="xspT")
                nc.sync.dma_start(
                    xspT_all[:].rearrange("p (c n) -> p c n", c=10),
                    xsp[r0:r0 + BLK, :], transpose=True)
                st["xspT"] = xspT_all
                st["xgT"] = xin.tile([D, BLK], BF16, tag="xgT", name="xgT")
                nc.sync.dma_start(st["xgT"][:], xg[r0:r0 + BLK, :], transpose=True)
                st["xfT"] = xin.tile([D, BLK], BF16, tag="xfT", name="xfT")
                nc.sync.dma_start(st["xfT"][:], xf[r0:r0 + BLK, :], transpose=True)
                return st

            def phase1(st):
                z_ps = []
                zs = psp.tile([D, BLK], F32, tag="ps")
                for c in range(10):
                    mm(zs[:], W["wspT"][:, c * D:(c + 1) * D],
                       st["xspT"][:, c * BLK:(c + 1) * BLK],
                       start=(c == 0), stop=(c == 9))
                z_ps.append(zs)
                for i, key in ((0, "xgT"), (1, "xfT")):
                    zt = psp.tile([D, BLK], F32, tag="ps")
                    mm(zt[:], W["wgfT"][:, i * D:(i + 1) * D], st[key][:])
                    z_ps.append(zt)
                zsb, sqs = [], []
                for n in range(NB):
                    z_sb = wk_.tile([D, BLK], BF16, tag=f"zsb{n}", bufs=1)
                    nc.scalar.activation(z_sb[:], z_ps[n][:], AF.Identity,
                                         bias=W["bc"][:, n:n + 1])
                    zsb.append(z_sb)
                    sq = wk_.tile([D, BLK], BF16, tag=f"sq1_{n}", bufs=1)
                    nc.vector.tensor_tensor(sq[:], z_sb[:], z_sb[:], AL.mult)
                    sqs.append(sq)
                bcast = ln_pack(sqs, "p1")
                P = []
                for n in range(NB):
                    rb_ps = bcast(n)
                    p_ = wk_.tile([D, BLK], BF16, tag=f"P{n}")
                    nc.vector.tensor_tensor(p_[:], zsb[n][:], rb_ps[:], AL.mult)
                    nc.vector.tensor_scalar_add(p_[:], p_[:], W["emb"][:, n:n + 1])
                    P.append(p_)
                st["P"] = P
                dP = []
                for n in range(NB):
                    s0, s1 = KV_IDX[n]
                    dp = wk_.tile([D, BLK], BF16, tag=f"dP{n}", bufs=2)
                    nc.vector.tensor_tensor(dp[:], P[s0][:], P[s1][:], AL.subtract)
                    dP.append(dp)
                st["dP"] = dP

            def phase2a(st):
                """attention: tanh-sigmoid, out-proj via 4-matmul PSUM
                accumulation; u = cast(o_ps + ob2)."""
                P, dP = st["P"], st["dP"]
                us, squ = [], []
                for n in range(NB):
                    s0, s1 = KV_IDX[n]
                    q_ps = psp.tile([D, BLK], F32, tag="ps")
                    mm(q_ps[:], W["wqT"][:, n * D:(n + 1) * D], P[n][:])
                    dk_ps = psp.tile([D, BLK], F32, tag="ps")
                    mm(dk_ps[:], W["wkT"][:, n * D:(n + 1) * D], dP[n][:])
                    dv_ps = psp.tile([D, BLK], F32, tag="ps")
                    mm(dv_ps[:], W["wvhT"][:, n * D:(n + 1) * D], dP[n][:])

                    q_sb = wk_.tile([D, BLK], BF16, tag="qsb", bufs=1)
                    nc.scalar.activation(q_sb[:], q_ps[:], AF.Copy)
                    t0 = wk_.tile([D, BLK], BF16, tag="t0", bufs=1)
                    nc.vector.tensor_tensor(t0[:], q_sb[:], dk_ps[:], AL.mult)
                    d_ps = psp.tile([D, BLK], F32, tag="ps")
                    mm(d_ps[:], W["hsel"][:], t0[:])
                    th = wk_.tile([D, BLK], BF16, tag="th", bufs=1)
                    nc.scalar.activation(th[:], d_ps[:], AF.Tanh,
                                         scale=ISQ * 0.5,
                                         bias=W["zerov"][:, 0:1])
                    tp = wk_.tile([D, BLK], BF16, tag="tp", bufs=1)
                    nc.vector.tensor_tensor(tp[:], th[:], dv_ps[:], AL.mult)

                    o_ps = psp.tile([D, BLK], F32, tag="ps")
                    mm(o_ps[:], W["owT"][:, n * D:(n + 1) * D], tp[:],
                       start=True, stop=False)
                    mm(o_ps[:], W["ovhT"][:, n * D:(n + 1) * D], P[s0][:],
                       start=False, stop=False)
                    mm(o_ps[:], W["ovhT"][:, n * D:(n + 1) * D], P[s1][:],
                       start=False, stop=False)
                    mm(o_ps[:], ident[:], P[n][:], start=False, stop=True)
                    u = wk_.tile([D, BLK], BF16, tag=f"u{n}")
                    nc.scalar.activation(u[:], o_ps[:], AF.Identity,
                                         bias=W["ob2"][:, n:n + 1])
                    us.append(u)
                    sq = wk_.tile([D, BLK], BF16, tag=f"sq2_{n}", bufs=1)
                    nc.vector.tensor_tensor(sq[:], u[:], u[:], AL.mult)
                    squ.append(sq)
                st["u"] = us
                st["sq2"] = squ

            def phase2b(st):
                bcast = ln_pack(st["sq2"], "p2")
                x1 = []
                for n in range(NB):
                    rb_ps = bcast(n)
                    x1n = wk_.tile([D, BLK], BF16, tag=f"x1{n}")
                    nc.vector.tensor_tensor(x1n[:], st["u"][n][:], rb_ps[:],
                                            AL.mult)
                    x1.append(x1n)
                st["x1"] = x1

            def phase3a(st):
                x1 = st["x1"]
                hs_all = []
                hp = []
                for n in range(NB):
                    for c in range(2):
                        h_ps = psp.tile([D, BLK], F32, tag="ps")
                        mm(h_ps[:],
                           W["w1T"][:, n * FFN + c * D: n * FFN + (c + 1) * D],
                           x1[n][:])
                        hp.append(h_ps)
                for n in range(NB):
                    h_sb = []
                    for c in range(2):
                        hs_ = wk_.tile([D, BLK], BF16, tag=f"hsb{n}_{c}", bufs=1)
                        nc.scalar.activation(hs_[:], hp[2 * n + c][:], AF.Gelu,
                                             bias=W["b1"][:, 2 * n + c: 2 * n + c + 1])
                        h_sb.append(hs_)
                    hs_all.append(h_sb)
                st["hs"] = hs_all

            def phase3b(st):
                x1 = st["x1"]
                x2p, sq3 = [], []
                for n in range(NB):
                    h_sb = st["hs"][n]
                    f_ps = psp.tile([D, BLK], F32, tag="ps")
                    for c in range(2):
                        mm(f_ps[:], W["w2T"][:, (2 * n + c) * D:(2 * n + c + 1) * D],
                           h_sb[c][:], start=(c == 0), stop=False)
                    mm(f_ps[:], ident[:], x1[n][:], start=False, stop=True)
                    xp = wk_.tile([D, BLK], BF16, tag=f"x2p{n}", bufs=1)
                    nc.scalar.activation(xp[:], f_ps[:], AF.Identity,
                                         bias=W["b2c"][:, n:n + 1])
                    x2p.append(xp)
                    sq = wk_.tile([D, BLK], BF16, tag=f"sq3_{n}", bufs=1)
                    nc.vector.tensor_tensor(sq[:], xp[:], xp[:], AL.mult)
                    sq3.append(sq)
                bcast = ln_pack(sq3, "p3")
                x2 = []
                for n in range(NB):
                    rb_ps = bcast(n)
                    x2n = wk_.tile([D, BLK], BF16, tag=f"x2{n}")
                    nc.vector.tensor_tensor(x2n[:], x2p[n][:], rb_ps[:], AL.mult)
                    x2.append(x2n)
                st["x2"] = x2

            def phase4a(st):
                x2 = st["x2"]
                g_ps = psp.tile([NB, BLK], F32, tag="ps")
                for n in range(NB):
                    mm(g_ps[:], W["gwT"][:, n * NB:(n + 1) * NB], x2[n][:],
                       start=(n == 0), stop=(n == 2))
                e_sb = wk_.tile([NB, BLK], BF16, tag="esb", bufs=1)
                nc.scalar.activation(e_sb[:], g_ps[:], AF.Exp,
                                     bias=W["gateb"][:NB, 0:1])
                zb_ps = psp.tile([D, BLK], F32, tag="ps")
                mm(zb_ps[:], W["ones3"][:NB, :], e_sb[:])
                rz = wk_.tile([D, BLK], BF16, tag="rz", bufs=1)
                nc.vector.reciprocal(rz[:], zb_ps[:])
                mns = []
                for n in range(NB):
                    eb_ps = psp.tile([D, BLK], F32, tag="ps")
                    mm(eb_ps[:], W["esel"][:NB, n * D:(n + 1) * D], e_sb[:])
                    mn = wk_.tile([D, BLK], BF16, tag=f"mn{n}", bufs=2)
                    nc.vector.tensor_tensor(mn[:], x2[n][:], eb_ps[:], AL.mult)
                    mns.append(mn)
                st["mn"] = mns
                st["rz"] = rz

            def phase4b(st, b):
                r0 = (b % nblk) * BLK
                mns, rz = st["mn"], st["rz"]
                acc = wk_.tile([D, BLK], BF16, tag="macc", bufs=1)
                nc.vector.tensor_tensor(acc[:], mns[0][:], mns[1][:], AL.add)
                acc2 = wk_.tile([D, BLK], BF16, tag="macc2", bufs=1)
                nc.vector.tensor_tensor(acc2[:], acc[:], mns[2][:], AL.add)
                fused = wk_.tile([D, BLK], BF16, tag="fused", bufs=1)
                nc.vector.tensor_tensor(fused[:], acc2[:], rz[:], AL.mult)

                ob_ps = psp.tile([D, BLK], BF16, tag="ps")
                for j in range(BLK // D):
                    nc.tensor.matmul(ob_ps[:, j * D:(j + 1) * D],
                                     fused[:, j * D:(j + 1) * D],
                                     ident[:], is_transpose=True)
                ob_sb = outp.tile([D, BLK], F32, tag="ob")
                nc.vector.tensor_copy(ob_sb[:], ob_ps[:])
                nc.gpsimd.dma_start(
                    out[r0:r0 + BLK, :].rearrange("(j p) k -> p j k", p=D),
                    ob_sb[:].rearrange("p (j k) -> p j k", j=BLK // D))

            # software-pipelined emission; G-set ACT ops (gelu, tanh) are
            # emitted at tick head, E-set ops (ln/exp) after, to bound ACT
            # table reloads at ~2 per tick
            total = nblk * repeat
            bstate = {}
            for t in range(total + 4):
                if 0 <= t - 3 < total:
                    phase3a(bstate[t - 3])
                if 0 <= t - 2 < total:
                    phase2a(bstate[t - 2])
                if 0 <= t - 2 < total:
                    phase2b(bstate[t - 2])
                if 0 <= t - 4 < total:
                    phase4a(bstate[t - 4])
                if 0 <= t - 1 < total:
                    phase1(bstate[t - 1])
                if 0 <= t - 3 < total:
                    phase3b(bstate[t - 3])
                if 0 <= t - 4 < total:
                    phase4b(bstate.pop(t - 4), t - 4)
                if t < total:
                    bstate[t] = phase0(t)
    _fix_wait_overflow(nc)
    return nc


def kernel(**inputs):
    _patch_tile_drain()
    B = inputs["x_spatial"].shape[0]
    Bc = B // NCORES
    w = prep_weights(inputs)
    nc = build_program(Bc)
    xb = {k: np.ascontiguousarray(inputs[k]).astype(NPBF)
          for k in ("x_spatial", "x_gradient", "x_frequency")}
    in_maps = []
    for c in range(NCORES):
        m = dict(w)
        for k in ("x_spatial", "x_gradient", "x_frequency"):
            m[k] = np.ascontiguousarray(xb[k][c * Bc:(c + 1) * Bc])
        in_maps.append(m)
    res = run_bass_kernel_spmd(nc, in_maps, list(range(NCORES)))
    return np.concatenate([res.results[c]["out"] for c in range(NCORES)], axis=0)


# revision 6
# speedup vs baseline: 1.1286x; 1.1286x over previous
"""Trainium2 Bass kernel for nn_CMAF (cross-modal attention fusion block).

Layout: feature-major activations on-chip — every tile is
[128 features (partitions) x 1024 samples (free)], so all matmuls are
weight-stationary bf16 with the batch as the moving free dimension.
Inputs are pre-cast to bf16 host-side and loaded straight into
feature-major SBUF via DMA-transpose (2-byte xbar path).

Engine-balance design (v3):
 - 2-way attention softmax = sigmoid((s0-s1)*ISQ) computed as
   0.5 + 0.5*tanh(d*ISQ/2); the 0.5 factors and the out-projection are
   folded into the weights host-side:
     o = ow @ (th (*) dv') + OVH @ P[s0] + OVH @ P[s1] + I @ P[n]
   with dv' = (0.5 Wv) dP, OVH = ow (0.5 Wv).  Residual adds ride the
   PSUM accumulator (identity matmuls) so the PSUM->SBUF landing is a
   single ScalarE cast with the bias folded in.
 - LayerNorm variance streams are packed [3, BLK]: three selector-column
   matmuls accumulate the per-branch variances into one PSUM tile, one
   Ln + one Exp (rsqrt via the natural_log_exp table set) serve all
   three branches, and tiny matmuls broadcast 1/sigma back to 128
   partitions.
 - ACT table sets: per-tick emission groups the gelu/tanh ops (set G)
   ahead of the ln/exp ops (set E) to bound table reloads at ~2/tick.
 - FFN runs on materialized x1, so Gelu reads its matmul PSUM directly.

LayerNorm mean subtraction is folded into the weights host-side
(centering matrix C = I - 11^T/128 on each producing linear layer).

Data parallel over 8 NeuronCores: 8192 samples each.
"""

import numpy as np
import ml_dtypes

import concourse.bass as bass
import concourse.mybir as mybir
from concourse.tile import TileContext
from concourse.vector_clock import ScopedClock
from concourse.bass_utils import run_bass_kernel_spmd

F32 = mybir.dt.float32
BF16 = mybir.dt.bfloat16
AL = mybir.AluOpType
AF = mybir.ActivationFunctionType
NPBF = ml_dtypes.bfloat16

D = 128
SP = 1280
FFN = 256
NB = 3
DH = 32
KV_IDX = ((1, 2), (0, 2), (0, 1))
NCORES = 8
BLK = 1024
MMN = 512
PSBUFS = 4
WB = 1
EPS = 1e-5
ISQ = float(1.0 / np.sqrt(DH))


def _patch_tile_drain():
    """walrus here rejects >4 sem waits on one instruction; Tile's tail
    drain carries one wait per logical proc.  Re-emit them as standalone
    wait_ge instructions ahead of the drain."""
    TC = TileContext
    if getattr(TC, "_drain_patched", False):
        return

    def patched(self, tick_clock, wait_clock):
        nop_inst = self.nc.sync.nop()
        wait_clock.add_sem_waits(
            nop_inst.ins, ScopedClock({None: tick_clock.global_clock})
        )
        d = nop_inst.ins
        si = d.sync_info
        waits = list(si.on_wait) if si is not None else []
        if len(waits) > 4:
            si.on_wait = []
            d.sync_info = si
            name2sem = {s.name: s for s in self.sems.allocated().values()}
            for w in waits:
                sem = name2sem.get(w.ant_name)
                if sem is None:
                    raise RuntimeError(f"drain patch: unknown sem {w.ant_name}")
                self.nc.sync.wait_ge(sem, w.wait_value)
        self.nc.sync.drain()
        self.nc.all_engine_barrier()
        popped = self.nc._tile_sem_poison_stack.pop()
        assert popped is self._sem_poison
        self.nc.clear_and_free_semaphores(list(self.sems.allocated().values()))
        self.nc.all_engine_barrier()

    TC._drain_and_barrier = patched
    TC._drain_patched = True


def _fix_wait_overflow(nc):
    """walrus enforces per-opcode caps on sync-wait commands attached to
    one instruction (DmaTransposeAnt: 1, others: ~4).  Move the excess
    onto same-engine NOPs inserted immediately before the instruction."""
    LIMITS = {}
    DEFAULT_LIM = 1
    for fn in nc.m.functions:
        for bb in fn.blocks:
            insts = list(bb.instructions)
            out = []
            changed = False
            for inst in insts:
                si = getattr(inst, "sync_info", None)
                w = list(si.on_wait) if si is not None and si.on_wait else []
                lim = LIMITS.get(type(inst).__name__, DEFAULT_LIM)
                if len(w) > lim:
                    excess = w[lim:]
                    keep = w[:lim]
                    eng = nc.engines[inst.engine]
                    nops = []
                    for i in range(0, len(excess), 1):
                        chunk = excess[i:i + 1]
                        nop_bi = eng.nop()
                        nop_inst = nop_bi.ins
                        cb = nc.cur_bb.bb
                        cb.instructions = [x for x in cb.instructions
                                           if x.name != nop_inst.name]
                        import bass_rust
                        nop_inst.sync_info = bass_rust.SyncInfo(
                            on_wait=chunk, on_update=[])
                        nops.append(nop_inst)
                    si.on_wait = keep
                    inst.sync_info = si
                    out.extend(nops)
                    changed = True
                out.append(inst)
            if changed:
                bb.instructions = out


def prep_weights(inp):
    """Host-side prep of all weights into SBUF layouts. bf16 for matmul
    operands, fp32 for per-partition bias vectors."""
    f64 = np.float64
    C = np.eye(D, dtype=f64) - 1.0 / D

    def bf(a):
        return np.ascontiguousarray(a.astype(np.float32)).astype(NPBF)

    def f32(a):
        return np.ascontiguousarray(a, dtype=np.float32)

    w = {}
    wsp = C @ inp["proj_w_spatial"].astype(f64)            # [128,1280]
    w["wspT"] = bf(np.transpose(wsp.reshape(D, 10, D), (2, 1, 0)).reshape(D, 10 * D))
    wgf = np.stack([C @ inp["proj_w_gf"][i].astype(f64) for i in range(2)])
    w["wgfT"] = bf(np.transpose(wgf, (2, 0, 1)).reshape(D, 2 * D))
    w["bc"] = f32(C @ inp["proj_b"].astype(f64).T)         # [128,3]
    w["emb"] = f32(inp["mod_emb"].T)

    ipw = inp["in_proj_w"].astype(f64)                     # [3, 384, 128]
    wq, wk, wv = ipw[:, :D], ipw[:, D:2 * D], ipw[:, 2 * D:]
    w["wqT"] = bf(np.transpose(wq, (2, 0, 1)).reshape(D, NB * D))
    w["wkT"] = bf(np.transpose(wk, (2, 0, 1)).reshape(D, NB * D))
    wvh = 0.5 * wv                                         # fold softmax 0.5
    w["wvhT"] = bf(np.transpose(wvh, (2, 0, 1)).reshape(D, NB * D))
    ow = np.stack([C @ inp["out_proj_w"][n].astype(f64) for n in range(NB)])
    w["owT"] = bf(np.transpose(ow, (2, 0, 1)).reshape(D, NB * D))
    ovh = np.stack([ow[n] @ wvh[n] for n in range(NB)])    # ow (0.5 Wv)
    w["ovhT"] = bf(np.transpose(ovh, (2, 0, 1)).reshape(D, NB * D))
    ob2 = np.stack([
        C @ inp["out_proj_b"][n].astype(f64)
        - inp["mod_emb"][n].astype(f64).mean()
        for n in range(NB)])
    w["ob2"] = f32(ob2.T)

    w1 = inp["ffn_w1"].astype(f64)                         # [3, 256, 128]
    w["w1T"] = bf(np.transpose(w1, (2, 0, 1)).reshape(D, NB * FFN))
    w["b1"] = f32(inp["ffn_b1"].reshape(NB * 2, D).T)      # [128, 6]
    w2 = np.stack([C @ inp["ffn_w2"][n].astype(f64) for n in range(NB)])
    w2c = w2.reshape(NB, D, 2, D)                          # [n, j, c, p]
    w["w2T"] = bf(np.transpose(w2c, (3, 0, 2, 1)).reshape(D, NB * 2 * D))
    b2c = np.stack([C @ inp["ffn_b2"][n].astype(f64) for n in range(NB)])
    w["b2c"] = f32(b2c.T)

    gw = inp["gate_w"].astype(f64).reshape(NB, NB, D)      # [j, n, p]
    w["gwT"] = bf(np.transpose(gw, (2, 1, 0)).reshape(D, NB * NB))
    w["gateb"] = f32(inp["gate_b"].reshape(NB, 1))

    # stats selector: column n = 1/D -> variance of branch n lands on
    # PSUM partition n (3 accumulating matmuls share one [3, BLK] tile)
    st = np.zeros((D, NB * NB), dtype=np.float32)
    for n in range(NB):
        st[:, NB * n + n] = 1.0 / D
    w["statsT"] = bf(st)
    # broadcast selector: row n ones -> rb3[n] broadcast to 128 partitions
    bsel = np.zeros((NB, NB * D), dtype=np.float32)
    for n in range(NB):
        bsel[n, n * D:(n + 1) * D] = 1.0
    w["bselT"] = bf(bsel)

    hs = np.zeros((D, D), dtype=np.float32)
    for h in range(4):
        hs[h * DH:(h + 1) * DH, h * DH:(h + 1) * DH] = 1.0
    w["hsel"] = bf(hs)
    w["ones3"] = bf(np.ones((NB, D)))
    esel = np.zeros((NB, NB * D), dtype=np.float32)
    for n in range(NB):
        esel[n, n * D:(n + 1) * D] = 1.0
    w["esel"] = bf(esel)
    w["ident"] = bf(np.eye(D))
    w["epsv"] = np.full((D, 1), EPS, dtype=np.float32)
    w["zerov"] = np.zeros((D, 1), dtype=np.float32)

    assert np.allclose(inp["proj_ln_g"], 1) and np.allclose(inp["proj_ln_b"], 0)
    assert np.allclose(inp["attn_ln_g"], 1) and np.allclose(inp["attn_ln_b"], 0)
    assert np.allclose(inp["ffn_ln_g"], 1) and np.allclose(inp["ffn_ln_b"], 0)
    assert np.allclose(inp["in_proj_b"], 0)
    return w


WEIGHT_SPECS = {
    "wspT": ((D, 10 * D), BF16), "wgfT": ((D, 2 * D), BF16),
    "bc": ((D, NB), F32), "emb": ((D, NB), F32),
    "wqT": ((D, NB * D), BF16), "wkT": ((D, NB * D), BF16),
    "wvhT": ((D, NB * D), BF16), "owT": ((D, NB * D), BF16),
    "ovhT": ((D, NB * D), BF16),
    "ob2": ((D, NB), F32),
    "w1T": ((D, NB * FFN), BF16), "b1": ((D, NB * 2), F32),
    "w2T": ((D, NB * 2 * D), BF16), "b2c": ((D, NB), F32),
    "gwT": ((D, NB * NB), BF16), "gateb": ((NB, 1), F32),
    "statsT": ((D, NB * NB), BF16), "bselT": ((NB, NB * D), BF16),
    "hsel": ((D, D), BF16),
    "ones3": ((NB, D), BF16), "esel": ((NB, NB * D), BF16),
    "ident": ((D, D), BF16),
    "epsv": ((D, 1), F32), "zerov": ((D, 1), F32),
}


def build_program(Bc, repeat=1):
    nc = bass.Bass()
    xsp = nc.dram_tensor("x_spatial", [Bc, SP], BF16, kind="ExternalInput")
    xg = nc.dram_tensor("x_gradient", [Bc, D], BF16, kind="ExternalInput")
    xf = nc.dram_tensor("x_frequency", [Bc, D], BF16, kind="ExternalInput")
    wd = {k: nc.dram_tensor(k, list(s[0]), s[1], kind="ExternalInput")
          for k, s in WEIGHT_SPECS.items()}
    out = nc.dram_tensor("out", [Bc, D], F32, kind="ExternalOutput")

    nblk = Bc // BLK
    assert Bc % BLK == 0

    with TileContext(nc) as tc, nc.allow_low_precision(reason="bf16 kernel"):
        with (
            tc.tile_pool(name="wp", bufs=1) as wp,
            tc.tile_pool(name="xin", bufs=2) as xin,
            tc.tile_pool(name="work", bufs=2) as wk_,
            tc.tile_pool(name="outp", bufs=1) as outp,
            tc.tile_pool(name="ps", bufs=PSBUFS, space="PSUM") as psp,
        ):
            W = {}
            for k, s in WEIGHT_SPECS.items():
                W[k] = wp.tile(list(s[0]), s[1], tag=k, name=k)
                nc.gpsimd.dma_start(W[k][:], wd[k][:])
            ident = W["ident"]

            def mm(out_ap, lhsT, rhs, start=True, stop=True):
                for h in range(BLK // MMN):
                    nc.tensor.matmul(out_ap[:, h * MMN:(h + 1) * MMN], lhsT,
                                     rhs[:, h * MMN:(h + 1) * MMN],
                                     start=start, stop=stop)

            def ln_pack(sqs, tag):
                """sqs: 3 bf16 [D, BLK] squared tiles -> per-branch 1/sigma
                broadcast tiles ([D, BLK] f32 PSUM), lazily per branch."""
                var3 = psp.tile([NB, BLK], F32, tag="ps")
                for n in range(NB):
                    mm(var3[:], W["statsT"][:, NB * n:NB * (n + 1)], sqs[n][:],
                       start=(n == 0), stop=(n == NB - 1))
                lnv = wk_.tile([NB, BLK], F32, tag="lnv", bufs=WB)
                nc.scalar.activation(lnv[:], var3[:], AF.Ln,
                                     bias=W["epsv"][:NB, 0:1])
                rb3 = wk_.tile([NB, BLK], BF16, tag="rb3", bufs=2)
                nc.scalar.activation(rb3[:], lnv[:], AF.Exp, scale=-0.5,
                                     bias=W["zerov"][:NB, 0:1])

                def bcast(n):
                    rb_ps = psp.tile([D, BLK], F32, tag="ps")
                    mm(rb_ps[:], W["bselT"][:NB, n * D:(n + 1) * D], rb3[:])
                    return rb_ps
                return bcast

            def phase0(b):
                r0 = (b % nblk) * BLK
                st = {}
                xspT_all = xin.tile([D, 10 * BLK], BF16, tag="xspT")
                nc.sync.dma_start(
                    xspT_all[:].rearrange("p (c n) -> p c n", c=10),
                    xsp[r0:r0 + BLK, :], transpose=True)
                st["xspT"] = xspT_all
                st["xgT"] = xin.tile([D, BLK], BF16, tag="xgT", name="xgT")
                nc.sync.dma_start(st["xgT"][:], xg[r0:r0 + BLK, :], transpose=True)
                st["xfT"] = xin.tile([D, BLK], BF16, tag="xfT", name="xfT")
                nc.sync.dma_start(st["xfT"][:], xf[r0:r0 + BLK, :], transpose=True)
                return st

            def phase1(st):
                z_ps = []
                zs = psp.tile([D, BLK], F32, tag="ps")
                for c in range(10):
                    mm(zs[:], W["wspT"][:, c * D:(c + 1) * D],
                       st["xspT"][:, c * BLK:(c + 1) * BLK],
                       start=(c == 0), stop=(c == 9))
                z_ps.append(zs)
                for i, key in ((0, "xgT"), (1, "xfT")):
                    zt = psp.tile([D, BLK], F32, tag="ps")
                    mm(zt[:], W["wgfT"][:, i * D:(i + 1) * D], st[key][:])
                    z_ps.append(zt)
                zsb, sqs = [], []
                for n in range(NB):
                    z_sb = wk_.tile([D, BLK], BF16, tag=f"zsb{n}", bufs=WB)
                    nc.scalar.activation(z_sb[:], z_ps[n][:], AF.Identity,
                                         bias=W["bc"][:, n:n + 1])
                    zsb.append(z_sb)
                    sq = wk_.tile([D, BLK], BF16, tag=f"sq1_{n}", bufs=WB)
                    nc.vector.tensor_tensor(sq[:], z_sb[:], z_sb[:], AL.mult)
                    sqs.append(sq)
                bcast = ln_pack(sqs, "p1")
                P = []
                for n in range(NB):
                    rb_ps = bcast(n)
                    p_ = wk_.tile([D, BLK], BF16, tag=f"P{n}")
                    nc.vector.tensor_tensor(p_[:], zsb[n][:], rb_ps[:], AL.mult)
                    nc.vector.tensor_scalar_add(p_[:], p_[:], W["emb"][:, n:n + 1])
                    P.append(p_)
                st["P"] = P
                dP = []
                for n in range(NB):
                    s0, s1 = KV_IDX[n]
                    dp = wk_.tile([D, BLK], BF16, tag=f"dP{n}", bufs=2)
                    nc.vector.tensor_tensor(dp[:], P[s0][:], P[s1][:], AL.subtract)
                    dP.append(dp)
                st["dP"] = dP

            def phase2a(st):
                """attention: tanh-sigmoid, out-proj via 4-matmul PSUM
                accumulation; u = cast(o_ps + ob2)."""
                P, dP = st["P"], st["dP"]
                us, squ = [], []
                for n in range(NB):
                    s0, s1 = KV_IDX[n]
                    q_ps = psp.tile([D, BLK], F32, tag="ps")
                    mm(q_ps[:], W["wqT"][:, n * D:(n + 1) * D], P[n][:])
                    dk_ps = psp.tile([D, BLK], F32, tag="ps")
                    mm(dk_ps[:], W["wkT"][:, n * D:(n + 1) * D], dP[n][:])
                    dv_ps = psp.tile([D, BLK], F32, tag="ps")
                    mm(dv_ps[:], W["wvhT"][:, n * D:(n + 1) * D], dP[n][:])

                    q_sb = wk_.tile([D, BLK], BF16, tag="qsb", bufs=WB)
                    nc.scalar.activation(q_sb[:], q_ps[:], AF.Copy)
                    t0 = wk_.tile([D, BLK], BF16, tag="t0", bufs=WB)
                    nc.vector.tensor_tensor(t0[:], q_sb[:], dk_ps[:], AL.mult)
                    d_ps = psp.tile([D, BLK], F32, tag="ps")
                    mm(d_ps[:], W["hsel"][:], t0[:])
                    th = wk_.tile([D, BLK], BF16, tag="th", bufs=WB)
                    nc.scalar.activation(th[:], d_ps[:], AF.Tanh,
                                         scale=ISQ * 0.5,
                                         bias=W["zerov"][:, 0:1])
                    tp = wk_.tile([D, BLK], BF16, tag="tp", bufs=WB)
                    nc.vector.tensor_tensor(tp[:], th[:], dv_ps[:], AL.mult)

                    o_ps = psp.tile([D, BLK], F32, tag="ps")
                    mm(o_ps[:], W["owT"][:, n * D:(n + 1) * D], tp[:],
                       start=True, stop=False)
                    mm(o_ps[:], W["ovhT"][:, n * D:(n + 1) * D], P[s0][:],
                       start=False, stop=False)
                    mm(o_ps[:], W["ovhT"][:, n * D:(n + 1) * D], P[s1][:],
                       start=False, stop=False)
                    mm(o_ps[:], ident[:], P[n][:], start=False, stop=True)
                    u = wk_.tile([D, BLK], BF16, tag=f"u{n}")
                    nc.scalar.activation(u[:], o_ps[:], AF.Identity,
                                         bias=W["ob2"][:, n:n + 1])
                    us.append(u)
                    sq = wk_.tile([D, BLK], BF16, tag=f"sq2_{n}", bufs=WB)
                    nc.vector.tensor_tensor(sq[:], u[:], u[:], AL.mult)
                    squ.append(sq)
                st["u"] = us
                st["sq2"] = squ

            def phase2b(st):
                bcast = ln_pack(st["sq2"], "p2")
                x1 = []
                for n in range(NB):
                    rb_ps = bcast(n)
                    x1n = wk_.tile([D, BLK], BF16, tag=f"x1{n}")
                    nc.vector.tensor_tensor(x1n[:], st["u"][n][:], rb_ps[:],
                                            AL.mult)
                    x1.append(x1n)
                st["x1"] = x1

            def phase3a(st):
                x1 = st["x1"]
                hs_all = []
                hp = []
                for n in range(NB):
                    for c in range(2):
                        h_ps = psp.tile([D, BLK], F32, tag="ps")
                        mm(h_ps[:],
                           W["w1T"][:, n * FFN + c * D: n * FFN + (c + 1) * D],
                           x1[n][:])
                        hp.append(h_ps)
                for n in range(NB):
                    h_sb = []
                    for c in range(2):
                        hs_ = wk_.tile([D, BLK], BF16, tag=f"hsb{n}_{c}", bufs=WB)
                        nc.scalar.activation(hs_[:], hp[2 * n + c][:], AF.Gelu,
                                             bias=W["b1"][:, 2 * n + c: 2 * n + c + 1])
                        h_sb.append(hs_)
                    hs_all.append(h_sb)
                st["hs"] = hs_all

            def phase3b(st):
                x1 = st["x1"]
                x2p, sq3 = [], []
                for n in range(NB):
                    h_sb = st["hs"][n]
                    f_ps = psp.tile([D, BLK], F32, tag="ps")
                    for c in range(2):
                        mm(f_ps[:], W["w2T"][:, (2 * n + c) * D:(2 * n + c + 1) * D],
                           h_sb[c][:], start=(c == 0), stop=False)
                    mm(f_ps[:], ident[:], x1[n][:], start=False, stop=True)
                    xp = wk_.tile([D, BLK], BF16, tag=f"x2p{n}", bufs=WB)
                    nc.scalar.activation(xp[:], f_ps[:], AF.Identity,
                                         bias=W["b2c"][:, n:n + 1])
                    x2p.append(xp)
                    sq = wk_.tile([D, BLK], BF16, tag=f"sq3_{n}", bufs=WB)
                    nc.vector.tensor_tensor(sq[:], xp[:], xp[:], AL.mult)
                    sq3.append(sq)
                bcast = ln_pack(sq3, "p3")
                x2 = []
                for n in range(NB):
                    rb_ps = bcast(n)
                    x2n = wk_.tile([D, BLK], BF16, tag=f"x2{n}")
                    nc.vector.tensor_tensor(x2n[:], x2p[n][:], rb_ps[:], AL.mult)
                    x2.append(x2n)
                st["x2"] = x2

            def phase4a(st):
                x2 = st["x2"]
                g_ps = psp.tile([NB, BLK], F32, tag="ps")
                for n in range(NB):
                    mm(g_ps[:], W["gwT"][:, n * NB:(n + 1) * NB], x2[n][:],
                       start=(n == 0), stop=(n == 2))
                e_sb = wk_.tile([NB, BLK], BF16, tag="esb", bufs=WB)
                nc.scalar.activation(e_sb[:], g_ps[:], AF.Exp,
                                     bias=W["gateb"][:NB, 0:1])
                zb_ps = psp.tile([D, BLK], F32, tag="ps")
                mm(zb_ps[:], W["ones3"][:NB, :], e_sb[:])
                rz = wk_.tile([D, BLK], BF16, tag="rz", bufs=WB)
                nc.vector.reciprocal(rz[:], zb_ps[:])
                mns = []
                for n in range(NB):
                    eb_ps = psp.tile([D, BLK], F32, tag="ps")
                    mm(eb_ps[:], W["esel"][:NB, n * D:(n + 1) * D], e_sb[:])
                    mn = wk_.tile([D, BLK], BF16, tag=f"mn{n}", bufs=2)
                    nc.vector.tensor_tensor(mn[:], x2[n][:], eb_ps[:], AL.mult)
                    mns.append(mn)
                st["mn"] = mns
                st["rz"] = rz

            def phase4b(st, b):
                r0 = (b % nblk) * BLK
                mns, rz = st["mn"], st["rz"]
                acc = wk_.tile([D, BLK], BF16, tag="macc", bufs=WB)
                nc.vector.tensor_tensor(acc[:], mns[0][:], mns[1][:], AL.add)
                acc2 = wk_.tile([D, BLK], BF16, tag="macc2", bufs=WB)
                nc.vector.tensor_tensor(acc2[:], acc[:], mns[2][:], AL.add)
                fused = wk_.tile([D, BLK], BF16, tag="fused", bufs=WB)
                nc.vector.tensor_tensor(fused[:], acc2[:], rz[:], AL.mult)

                ob_ps = psp.tile([D, BLK], BF16, tag="ps")
                for j in range(BLK // D):
                    nc.tensor.matmul(ob_ps[:, j * D:(j + 1) * D],
                                     fused[:, j * D:(j + 1) * D],
                                     ident[:], is_transpose=True)
                ob_sb = outp.tile([D, BLK], F32, tag="ob")
                nc.vector.tensor_copy(ob_sb[:], ob_ps[:])
                nc.gpsimd.dma_start(
                    out[r0:r0 + BLK, :].rearrange("(j p) k -> p j k", p=D),
                    ob_sb[:].rearrange("p (j k) -> p j k", j=BLK // D))

            # software-pipelined emission; G-set ACT ops (gelu, tanh) are
            # emitted at tick head, E-set ops (ln/exp) after, to bound ACT
            # table reloads at ~2 per tick
            total = nblk * repeat
            bstate = {}
            for t in range(total + 4):
                if 0 <= t - 3 < total:
                    phase3a(bstate[t - 3])
                if 0 <= t - 2 < total:
                    phase2a(bstate[t - 2])
                if 0 <= t - 2 < total:
                    phase2b(bstate[t - 2])
                if 0 <= t - 4 < total:
                    phase4a(bstate[t - 4])
                if 0 <= t - 1 < total:
                    phase1(bstate[t - 1])
                if 0 <= t - 3 < total:
                    phase3b(bstate[t - 3])
                if 0 <= t - 4 < total:
                    phase4b(bstate.pop(t - 4), t - 4)
                if t < total:
                    bstate[t] = phase0(t)
    _fix_wait_overflow(nc)
    return nc


def kernel(**inputs):
    _patch_tile_drain()
    B = inputs["x_spatial"].shape[0]
    Bc = B // NCORES
    w = prep_weights(inputs)
    nc = build_program(Bc)
    xb = {k: np.ascontiguousarray(inputs[k]).astype(NPBF)
          for k in ("x_spatial", "x_gradient", "x_frequency")}
    in_maps = []
    for c in range(NCORES):
        m = dict(w)
        for k in ("x_spatial", "x_gradient", "x_frequency"):
            m[k] = np.ascontiguousarray(xb[k][c * Bc:(c + 1) * Bc])
        in_maps.append(m)
    res = run_bass_kernel_spmd(nc, in_maps, list(range(NCORES)))
    return np.concatenate([res.results[c]["out"] for c in range(NCORES)], axis=0)
